# revision 8
# baseline (speedup 1.0000x reference)
"""DeepClusteringLoss on 8 TRN2 NeuronCores.

loss = -sum_b ||E_b^T Y_b||_F^2 / (mean_b ||E_b^T E_b||_F^2 + 1e-8)
with Y = V / (colsum(V) + 1e-8), E: (B, N, D), V: (B, N, S), N = F*T.

Sharding: data-parallel over batch (8 batches -> 8 cores). Each core
reduces its shard to a (110,110) Gram block matrix on-device; the host
extracts the 5 diagonal 22x22 blocks and combines per-batch scalars.

Device algorithm (per core), raw Bass (no Tile framework preamble):
  Host packs each row as 22 fp8e4m3 values [e_0..e_19, y_0*2^17,
  y_1*2^17] where y = v / (colsum(v)+1e-8) is normalized ON HOST
  (elementwise prep, like the fp8 cast itself).  The interleave makes
  E^T Y fall out of the same self-Gram matmul as E^T E.  N=257000 rows
  are zero-padded to 2010*128 = 257280.  The padded array is split into
  8 SWDGE DMA groups; group i is viewed as (128, m_i*22) fp8: partition
  p holds m_i consecutive 22-byte rows, one contiguous DRAM read per
  partition line.
  Matmuls contract over the 128 partitions: each slice covers 5 rows
  per partition (110 cols); stationary is the 110 cols + 18 slack bytes
  (to hit the compiler's NumWeights==128 FWL trigger), moving is the
  contiguous 110 cols.  All 402 slices PSUM-accumulate into one
  [128,110] bank; diagonal 22x22 blocks of rows 0..109 hold the
  full-batch Gram sums.
  Microbench-validated facts this layout leans on (see microbench*.py):
  back-to-back cadence for this slice shape is ~53ns warm at 2.4 GHz
  (64ns if the chip sits at 2.0 GHz); PSUM bank choice, accumulation,
  and FWL-vs-not change it by <2ns -- it is moving-column-bound; dummy
  warmup matmuls on UNINITIALIZED SBUF are safe and full speed, so no
  memset (tensor engine starts its HAM clock warmup immediately after
  the fixed ~7.4us engine preamble, and gpsimd starts SWDGE descriptor
  generation at the same time instead of serializing behind a memset).
  Group sizes ramp up so each group lands just before the tensor engine
  finishes the previous one (supply 8.9ns/chunk vs demand 12.8ns/chunk
  warm).  Output tail: DVE copies PSUM->SBUF, then vector+scalar
  engines issue the two halves of the 48KB output DMA concurrently
  (HWDGE descriptor generation ~0.6us each, serialized it would be
  ~1.2us).
"""

import sys

if "/opt/trn_rl_repo" not in sys.path:
    sys.path.insert(0, "/opt/trn_rl_repo")

from contextlib import ExitStack

import ml_dtypes
import numpy as np

import concourse.bass as bass
from concourse import mybir
from concourse.bass_utils import run_bass_kernel_spmd

# Problem geometry (hardcoded; see spec)
B, F, T, D, S = 8, 257, 1000, 20, 2
N = F * T  # 257000
CH = 22  # fp8 cols per row: [e0..e19, y0*SC, y1*SC]
P = 128  # SBUF partitions
C = 5  # rows per partition per matmul slice (C*CH = 110 <= 128 stationary)
BLK = C * CH  # 110
SC = 2.0**17  # host scale on y (keeps y*SC in fp8 normal range, max ~1)
SLACK = P - BLK  # 18 extra stationary cols to trigger FWL (NumWeights==128)

# Row-chunks (128 rows each) per DMA group; sum = 2010 -> NPAD = 257280.
# Each divisible by C.  Sized so group i+1 finishes landing just before
# the PE finishes group i: supply ~8.9 ns/chunk (SWDGE ~318 GB/s, P0
# clock) vs demand 12.8 ns/chunk warm (64 ns per 5-chunk slice), with
# the PE starting ~10.4us (first group landed) and cold until ~10.9us.
MS = [25, 35, 65, 70, 70, 70, 75, 110, 160, 235, 345, 515, 235]
NPAD = P * sum(MS)  # 257280
N_WARM = 11  # PE warm-up dummy matmuls (256 moving cols each, ~240ns
# cold): keep the PE busy from ~7.45us (preamble end) to ~10.4us
# (group-1 landing) so the HAM governor promotes 1.2 -> 2.4 GHz at
# ~10.9us without the real stream ever running cold for long.
WARM_MOV = 256
FP8 = ml_dtypes.float8_e4m3


def build_bass(ms=None, n_warm=N_WARM, n_cores=B):
    """Build the per-core raw-Bass SPMD program (same program on every
    core; only the input data differs)."""
    ms = list(MS if ms is None else ms)
    assert all(m % C == 0 for m in ms)
    npad = P * sum(ms)
    ngrp = len(ms)

    nc = bass.Bass("TRN2", debug=False, num_devices=n_cores)
    ev = nc.dram_tensor("ev", [npad, CH], mybir.dt.float8e4, kind="ExternalInput")
    out_g = nc.dram_tensor(
        "out_g", [BLK, BLK], mybir.dt.float32, kind="ExternalOutput"
    )

    # DRAM views per group: (128, m*CH), partition-major rows
    bases = np.cumsum([0] + ms).tolist()
    ev_views = [
        ev.ap()[P * bases[i] : P * bases[i + 1], :].rearrange(
            "(p m) d -> p (m d)", p=P
        )
        for i in range(ngrp)
    ]

    with ExitStack() as ctx:
        # +SLACK uninitialized bytes per buffer so the last slice's
        # stationary can borrow 18 cols (garbage only feeds PSUM rows
        # 110..127, never read)
        bufs = [
            ctx.enter_context(
                nc.sbuf_tensor(f"buf{i}", [P, m * CH + SLACK], mybir.dt.float8e4)
            )
            for i, m in enumerate(ms)
        ]
        scr = ctx.enter_context(
            nc.sbuf_tensor("scr", [P, WARM_MOV], mybir.dt.float8e4)
        )  # never written: uninitialized SBUF is fine for dummy matmuls
        gsb = ctx.enter_context(nc.sbuf_tensor("gsb", [BLK, BLK], mybir.dt.float32))
        gacc = ctx.enter_context(nc.psum_tensor("gacc", [P, BLK], mybir.dt.float32))
        warm_ps = ctx.enter_context(
            nc.psum_tensor("warm_ps", [P, WARM_MOV], mybir.dt.float32)
        )
        dma_sems = [
            ctx.enter_context(nc.semaphore(f"dma_sem{i}")) for i in range(ngrp)
        ]
        ten_sem = ctx.enter_context(nc.semaphore("ten_sem"))
        copy_sem = ctx.enter_context(nc.semaphore("copy_sem"))
        odma_sem = ctx.enter_context(nc.semaphore("odma_sem"))
        block = ctx.enter_context(nc.Block(no_gpsimd_drain=False))

        @block.gpsimd
        def _(g: bass.BassEngine):
            # SWDGE descriptor generation (~0.68us per group, serialized
            # on gpsimd; 16 SDMA engines then stream the groups in
            # issue order).
            for i in range(ngrp):
                g.dma_start(
                    out=bufs[i].ap()[:, : ms[i] * CH], in_=ev_views[i]
                ).then_inc(dma_sems[i], 16)


        @block.tensor
        def _(t: bass.BassEngine):
            # PE warm-up on uninitialized scratch: the HAM governor
            # promotes the clock after ~3.4us of unbroken execution;
            # overlap that with the DMA lead-in.  warm_ps is never read.
            for _ in range(n_warm):
                t.matmul(
                    warm_ps.ap(), scr.ap()[:, :P], scr.ap(), start=True, stop=True
                )
            total = sum(m // C for m in ms)
            gi = 0
            last = None
            for i, m in enumerate(ms):
                t.wait_ge(dma_sems[i], 16)
                buf = bufs[i]
                for j in range(m // C):
                    sta = buf.ap()[:, j * BLK : j * BLK + P]
                    mov = buf.ap()[:, j * BLK : (j + 1) * BLK]
                    last = t.matmul(
                        gacc.ap(),
                        sta,
                        mov,
                        start=(gi == 0),
                        stop=(gi == total - 1),
                    )
                    gi += 1
            last.then_inc(ten_sem, 1)

        @block.vector
        def _(v: bass.BassEngine):
            # DVE does the PSUM -> SBUF copy.
            v.wait_ge(ten_sem, 1)
            v.tensor_copy(gsb.ap(), gacc.ap()[:BLK, :]).then_inc(copy_sem, 3)

        @block.sync
        def _(s: bass.BassEngine):
            # First half of the output DMA.  No explicit wait on the
            # out-DMAs: the end-of-block DRAIN fences the HWDGE queues,
            # and the host fetches results after NEFF completion.
            s.wait_ge(copy_sem, 1)
            s.dma_start(
                out=out_g.ap()[:78, :], in_=gsb.ap()[:78, :]
            ).then_inc(odma_sem, 16)

        @block.scalar
        def _(s: bass.BassEngine):
            # Second half of the output DMA, issued concurrently.
            s.wait_ge(copy_sem, 2)
            s.dma_start(
                out=out_g.ap()[78:, :], in_=gsb.ap()[78:, :]
            ).then_inc(odma_sem, 16)

    return nc


def pack_inputs(embeddings, source_indicators, npad=NPAD):
    """(B,F,T,D)+(B,F,T,S) -> per-core padded interleaved (npad, 22) fp8.

    y = v / (colsum(v) + 1e-8) is normalized here (host-side elementwise
    prep, same spirit as the fp8 cast); scaled by SC=2^17 so the values
    sit in fp8 normal range (max ~1.0)."""
    b = embeddings.shape[0]
    n = embeddings.shape[1] * embeddings.shape[2]
    e = np.asarray(embeddings, dtype=np.float32).reshape(b, n, D)
    v = np.asarray(source_indicators, dtype=np.float32).reshape(b, n, S)
    y = v / (np.sum(v, axis=1, keepdims=True) + 1e-8)
    evp = np.zeros((b, npad, CH), dtype=FP8)
    evp[:, :n, :D] = e.astype(FP8)
    evp[:, :n, D:] = (y * SC).astype(FP8)
    return evp


def reduce_outputs(res):
    """Per-core raw output -> (G_b, EtY_b) in float64.

    The [110,110] Gram block matrix has the per-chunk sums in its 5
    diagonal 22x22 blocks; within each, [:20,:20] is E^T E and
    [:20,20:22] is E^T (Y*SC)."""
    out_g = np.asarray(res["out_g"], dtype=np.float64)
    g_b = np.zeros((D, D))
    ety_b = np.zeros((D, S))
    for c in range(C):
        blk = out_g[c * CH : (c + 1) * CH, c * CH : (c + 1) * CH]
        g_b += blk[:D, :D]
        ety_b += blk[:D, D:]
    return g_b, ety_b / SC


_NC_CACHE = {}


def _get_nc():
    if "nc" not in _NC_CACHE:
        _NC_CACHE["nc"] = build_bass()
    return _NC_CACHE["nc"]


def kernel(embeddings, source_indicators):
    evp = pack_inputs(embeddings, source_indicators)
    nc = _get_nc()
    in_maps = [{"ev": np.ascontiguousarray(evp[b])} for b in range(B)]
    results = run_bass_kernel_spmd(nc, in_maps, list(range(B))).results

    loss = 0.0
    norms = []
    for b in range(B):
        g_b, ety_b = reduce_outputs(results[b])
        loss += float(np.sum(ety_b * ety_b))
        norms.append(float(np.sum(g_b * g_b)))
    norm_term = float(np.mean(norms))
    return np.float32(-loss / (norm_term + 1e-8))


# revision 10
# speedup vs baseline: 1.0575x; 1.0575x over previous
"""DeepClusteringLoss on 8 TRN2 NeuronCores.

loss = -sum_b ||E_b^T Y_b||_F^2 / (mean_b ||E_b^T E_b||_F^2 + 1e-8)
with Y = V / (colsum(V) + 1e-8), E: (B, N, D), V: (B, N, S), N = F*T.

Sharding: data-parallel over batch (8 batches -> 8 cores). Each core
reduces its shard to a (110,110) Gram block matrix on-device; the host
extracts the 5 diagonal 22x22 blocks and combines per-batch scalars.

Device algorithm (per core), raw Bass (no Tile framework preamble):
  Host packs each row as 22 fp8e4m3 values [e_0..e_19, y_0*2^17,
  y_1*2^17] where y = v / (colsum(v)+1e-8) is normalized ON HOST
  (elementwise prep, like the fp8 cast itself).  The interleave makes
  E^T Y fall out of the same self-Gram matmul as E^T E.  N=257000 rows
  are zero-padded to 2010*128 = 257280.  The padded array is split into
  8 SWDGE DMA groups; group i is viewed as (128, m_i*22) fp8: partition
  p holds m_i consecutive 22-byte rows, one contiguous DRAM read per
  partition line.
  Matmuls contract over the 128 partitions: each slice covers 5 rows
  per partition (110 cols); stationary is the 110 cols + 18 slack bytes
  (to hit the compiler's NumWeights==128 FWL trigger), moving is the
  contiguous 110 cols.  All 402 slices PSUM-accumulate into one
  [128,110] bank; diagonal 22x22 blocks of rows 0..109 hold the
  full-batch Gram sums.
  Microbench-validated facts this layout leans on (see microbench*.py):
  back-to-back cadence for this slice shape is ~53ns warm at 2.4 GHz
  (64ns if the chip sits at 2.0 GHz); PSUM bank choice, accumulation,
  and FWL-vs-not change it by <2ns -- it is moving-column-bound; dummy
  warmup matmuls on UNINITIALIZED SBUF are safe and full speed, so no
  memset (tensor engine starts its HAM clock warmup immediately after
  the fixed ~7.4us engine preamble, and gpsimd starts SWDGE descriptor
  generation at the same time instead of serializing behind a memset).
  Group sizes ramp up so each group lands just before the tensor engine
  finishes the previous one (supply 8.9ns/chunk vs demand 12.8ns/chunk
  warm).  Output tail: DVE copies PSUM->SBUF, then vector+scalar
  engines issue the two halves of the 48KB output DMA concurrently
  (HWDGE descriptor generation ~0.6us each, serialized it would be
  ~1.2us).
"""

import sys

if "/opt/trn_rl_repo" not in sys.path:
    sys.path.insert(0, "/opt/trn_rl_repo")

from contextlib import ExitStack

import ml_dtypes
import numpy as np

import concourse.bass as bass
from concourse import mybir
from concourse.bass_utils import run_bass_kernel_spmd

# Problem geometry (hardcoded; see spec)
B, F, T, D, S = 8, 257, 1000, 20, 2
N = F * T  # 257000
CH = 22  # fp8 cols per row: [e0..e19, y0*SC, y1*SC]
P = 128  # SBUF partitions
C = 5  # rows per partition per matmul slice (C*CH = 110 <= 128 stationary)
BLK = C * CH  # 110
SC = 2.0**17  # host scale on y (keeps y*SC in fp8 normal range, max ~1)
SLACK = P - BLK  # 18 extra stationary cols to trigger FWL (NumWeights==128)

# Row-chunks (128 rows each) per DMA group; sum = 2010 -> NPAD = 257280.
# Each divisible by C.  Sized so group i+1 finishes landing just before
# the PE finishes group i: supply ~8.9 ns/chunk (SWDGE ~318 GB/s, P0
# clock) vs demand 12.8 ns/chunk warm (64 ns per 5-chunk slice), with
# the PE starting ~10.4us (first group landed) and cold until ~10.9us.
MS = [50, 100, 135, 145, 145, 150, 160, 165, 170, 175, 185, 190, 165, 75]
NPAD = P * sum(MS)  # 257280
N_WARM = 16  # PE warm-up dummy matmuls (256 moving cols each, ~240ns
# cold): keep the PE busy from ~7.45us (preamble end) to ~10.4us
# (group-1 landing) so the HAM governor promotes 1.2 -> 2.4 GHz at
# ~10.9us without the real stream ever running cold for long.
WARM_MOV = 256
FP8 = ml_dtypes.float8_e4m3


def build_bass(ms=None, n_warm=N_WARM, n_cores=B):
    """Build the per-core raw-Bass SPMD program (same program on every
    core; only the input data differs)."""
    ms = list(MS if ms is None else ms)
    assert all(m % C == 0 for m in ms)
    npad = P * sum(ms)
    ngrp = len(ms)

    nc = bass.Bass("TRN2", debug=False, num_devices=n_cores)
    ev = nc.dram_tensor("ev", [npad, CH], mybir.dt.float8e4, kind="ExternalInput")
    out_g = nc.dram_tensor(
        "out_g", [BLK, BLK], mybir.dt.float32, kind="ExternalOutput"
    )

    # DRAM views per group: (128, m*CH), partition-major rows
    bases = np.cumsum([0] + ms).tolist()
    ev_views = [
        ev.ap()[P * bases[i] : P * bases[i + 1], :].rearrange(
            "(p m) d -> p (m d)", p=P
        )
        for i in range(ngrp)
    ]

    with ExitStack() as ctx:
        # +SLACK uninitialized bytes per buffer so the last slice's
        # stationary can borrow 18 cols (garbage only feeds PSUM rows
        # 110..127, never read)
        bufs = [
            ctx.enter_context(
                nc.sbuf_tensor(f"buf{i}", [P, m * CH + SLACK], mybir.dt.float8e4)
            )
            for i, m in enumerate(ms)
        ]
        scr = ctx.enter_context(
            nc.sbuf_tensor("scr", [P, WARM_MOV], mybir.dt.float8e4)
        )  # never written: uninitialized SBUF is fine for dummy matmuls
        gsb = ctx.enter_context(nc.sbuf_tensor("gsb", [BLK, BLK], mybir.dt.float32))
        gacc = ctx.enter_context(nc.psum_tensor("gacc", [P, BLK], mybir.dt.float32))
        warm_ps = ctx.enter_context(
            nc.psum_tensor("warm_ps", [P, WARM_MOV], mybir.dt.float32)
        )
        dma_sems = [
            ctx.enter_context(nc.semaphore(f"dma_sem{i}")) for i in range(ngrp)
        ]
        ten_sem = ctx.enter_context(nc.semaphore("ten_sem"))
        copy_sem = ctx.enter_context(nc.semaphore("copy_sem"))
        odma_sem = ctx.enter_context(nc.semaphore("odma_sem"))
        block = ctx.enter_context(nc.Block(no_gpsimd_drain=False))

        @block.gpsimd
        def _(g: bass.BassEngine):
            # SWDGE descriptor generation (~0.68us per group, serialized
            # on gpsimd; 16 SDMA engines then stream the groups in
            # issue order).
            for i in range(ngrp):
                g.dma_start(
                    out=bufs[i].ap()[:, : ms[i] * CH], in_=ev_views[i]
                ).then_inc(dma_sems[i], 16)


        @block.tensor
        def _(t: bass.BassEngine):
            # PE warm-up on uninitialized scratch: the HAM governor
            # promotes the clock after ~3.4us of unbroken execution;
            # overlap that with the DMA lead-in.  warm_ps is never read.
            for _ in range(n_warm):
                t.matmul(
                    warm_ps.ap(), scr.ap()[:, :P], scr.ap(), start=True, stop=True
                )
            total = sum(m // C for m in ms)
            gi = 0
            last = None
            for i, m in enumerate(ms):
                t.wait_ge(dma_sems[i], 16)
                buf = bufs[i]
                for j in range(m // C):
                    sta = buf.ap()[:, j * BLK : j * BLK + P]
                    mov = buf.ap()[:, j * BLK : (j + 1) * BLK]
                    last = t.matmul(
                        gacc.ap(),
                        sta,
                        mov,
                        start=(gi == 0),
                        stop=(gi == total - 1),
                    )
                    gi += 1
            last.then_inc(ten_sem, 1)

        @block.vector
        def _(v: bass.BassEngine):
            # DVE does the PSUM -> SBUF copy.
            v.wait_ge(ten_sem, 1)
            v.tensor_copy(gsb.ap(), gacc.ap()[:BLK, :]).then_inc(copy_sem, 3)

        @block.sync
        def _(s: bass.BassEngine):
            # First half of the output DMA.  No explicit wait on the
            # out-DMAs: the end-of-block DRAIN fences the HWDGE queues,
            # and the host fetches results after NEFF completion.
            s.wait_ge(copy_sem, 1)
            s.dma_start(
                out=out_g.ap()[:78, :], in_=gsb.ap()[:78, :]
            ).then_inc(odma_sem, 16)

        @block.scalar
        def _(s: bass.BassEngine):
            # Second half of the output DMA, issued concurrently.
            s.wait_ge(copy_sem, 2)
            s.dma_start(
                out=out_g.ap()[78:, :], in_=gsb.ap()[78:, :]
            ).then_inc(odma_sem, 16)

    return nc


def pack_inputs(embeddings, source_indicators, npad=NPAD):
    """(B,F,T,D)+(B,F,T,S) -> per-core padded interleaved (npad, 22) fp8.

    y = v / (colsum(v) + 1e-8) is normalized here (host-side elementwise
    prep, same spirit as the fp8 cast); scaled by SC=2^17 so the values
    sit in fp8 normal range (max ~1.0)."""
    b = embeddings.shape[0]
    n = embeddings.shape[1] * embeddings.shape[2]
    e = np.asarray(embeddings, dtype=np.float32).reshape(b, n, D)
    v = np.asarray(source_indicators, dtype=np.float32).reshape(b, n, S)
    y = v / (np.sum(v, axis=1, keepdims=True) + 1e-8)
    evp = np.zeros((b, npad, CH), dtype=FP8)
    evp[:, :n, :D] = e.astype(FP8)
    evp[:, :n, D:] = (y * SC).astype(FP8)
    return evp


def reduce_outputs(res):
    """Per-core raw output -> (G_b, EtY_b) in float64.

    The [110,110] Gram block matrix has the per-chunk sums in its 5
    diagonal 22x22 blocks; within each, [:20,:20] is E^T E and
    [:20,20:22] is E^T (Y*SC)."""
    out_g = np.asarray(res["out_g"], dtype=np.float64)
    g_b = np.zeros((D, D))
    ety_b = np.zeros((D, S))
    for c in range(C):
        blk = out_g[c * CH : (c + 1) * CH, c * CH : (c + 1) * CH]
        g_b += blk[:D, :D]
        ety_b += blk[:D, D:]
    return g_b, ety_b / SC


_NC_CACHE = {}


def _get_nc():
    if "nc" not in _NC_CACHE:
        _NC_CACHE["nc"] = build_bass()
    return _NC_CACHE["nc"]


def kernel(embeddings, source_indicators):
    evp = pack_inputs(embeddings, source_indicators)
    nc = _get_nc()
    in_maps = [{"ev": np.ascontiguousarray(evp[b])} for b in range(B)]
    results = run_bass_kernel_spmd(nc, in_maps, list(range(B))).results

    loss = 0.0
    norms = []
    for b in range(B):
        g_b, ety_b = reduce_outputs(results[b])
        loss += float(np.sum(ety_b * ety_b))
        norms.append(float(np.sum(g_b * g_b)))
    norm_term = float(np.mean(norms))
    return np.float32(-loss / (norm_term + 1e-8))
